# revision 29
# baseline (speedup 1.0000x reference)
"""Trainium2 Bass kernel for the DeformationGraph problem.

Math: per batch b and vertex v,
    out[b,v,k] = sum_c W[v,c] * ( sum_d (X[b,v,d]-center[b,c,d]) * R[b,c,k,d]
                                  + center[b,c,k] + V_nodes[b,c,k] )
factors into a vertex-independent per-node affine map:
    t[b,c,k]   = center[b,c,k] + V_nodes[b,c,k] - sum_d center[b,c,d]*R[b,c,k,d]
    out[b,v,k] = sum_d X[b,v,d] * (W @ R[..,k,d])[v]  +  (W @ t[..,k])[v]
i.e. one (V,C)@(C,48) matmul Y = W @ G, then a per-vertex contraction of Y
with [X,1].  W/X/out are sharded over the vertex dim across 8 cores.

Precision: rel-err budget is 2e-2; a single bf16 term (W, G, x, and the
product tensor all bf16, fp32 accumulation) measures ~3e-3 end-to-end,
so no multi-term splits are used.

Per-core pipeline (vertex shard padded to 6272 = 6*1024 + 128):
  - PE: per 1024-vertex pair, y[0:64] and y[64:128] in one PSUM tile get
    (K=128 "A" + K=32 "B") accumulated matmuls (G-column layout
    j = k*16 + d*4 + b, d==3 = translation, cols 48:64 zero).
  - DVE: one [128,512] tensor_mul  s = y * xd2  (bf16 out to SBUF).
    xd2 is the compact per-vertex [X,1] table replicated 4x along
    partitions by SBUF->SBUF DMAs so the multiply is partition-tall
    (engine op cost scales with free-dim columns only).
  - PE again: the 4-way d-reduction runs as a 0/1 "reduction matmul"
    r[24, n] = RED^T @ s  (rows h*12 + k*4 + b), output PSUM, DMA'd
    straight to DRAM.  This keeps DVE at one op per 1024 vertices and
    leaves ACT/Pool free for DMA issue.
HBM traffic/core: ~1.6MB W_A + 0.4MB W_B + 0.2MB x + 0.3MB out = 2.5MB.
"""

import numpy as np
import ml_dtypes

import concourse.mybir as mybir
import concourse.tile as tile
from concourse import bacc
from concourse.bass_utils import run_bass_kernel_spmd

B, V, C = 4, 50000, 160
N_CORES = 8
VS = V // N_CORES            # 6250 vertices per core
VSP = 6272                   # padded shard: 6 pairs of 1024 + 128 tail
NPAIR = 6
PC = 3200                    # pair-col space: 6*512 + 128
F32 = mybir.dt.float32
BF16 = mybir.dt.bfloat16
NPBF16 = ml_dtypes.bfloat16

WCH = [(0, 1024), (1024, 4096), (4096, VSP)]   # wha DMA chunks (vertex cols)
BCH = [(0, 1024), (1024, PC)]                  # whb2 DMA chunks (pair cols)
N_WARM = 40                                    # N=128 ramp matmuls


def _locate(tiles, chunks, g0, width):
    for t, (c0, c1) in zip(tiles, chunks):
        if c0 <= g0 and g0 + width <= c1:
            return t, slice(g0 - c0, g0 - c0 + width)
    raise AssertionError(f"col range {g0}+{width} crosses chunk boundary")


def _build_bass():
    nc = bacc.Bacc()

    cst_d = nc.dram_tensor("cst", [128, 224], BF16, kind="ExternalInput")
    wha_d = nc.dram_tensor("wha", [128, VSP], BF16, kind="ExternalInput")
    whb_d = nc.dram_tensor("whb", [64, PC], BF16, kind="ExternalInput")
    xc_d = nc.dram_tensor("xc", [128, PC], BF16, kind="ExternalInput")
    outT = nc.dram_tensor("outT", [24, PC], F32, kind="ExternalOutput")

    with tile.TileContext(nc) as tc:
        with (
            tc.tile_pool(name="cpool", bufs=1) as cpool,
            tc.tile_pool(name="spool", bufs=3) as spool,
            tc.tile_pool(name="ypool", bufs=2, space="PSUM") as ypool,
            tc.tile_pool(name="rpool", bufs=2, space="PSUM") as rpool,
        ):
            cst = cpool.tile([128, 224], BF16)
            nc.sync.dma_start(out=cst[:], in_=cst_d[:])

            ghA = cst[:, 0:64]
            RED24 = cst[:, 64:88]
            ghB2 = cst[0:64, 96:224]   # block-diag [[G_B,0],[0,G_B]]

            # xd2: per-vertex [X,1] table, 4 copies along partitions so the
            # multiply runs partition-tall.  Rows h*64 + k*16 + (d*4+b).
            # Shipped fully inflated (800KB): chained on-chip doubling kept
            # losing the DMA-ordering race and gating the first multiply.
            # W streams spread across issue queues: first whb2 chunk on
            # sync (small, needed early), wha on scalar's HWDGE queue,
            # bulky whb2 chunk 1 on gpsimd's SWDGE (latency-tolerant).
            whb_t = []
            for i, (c0, c1) in enumerate(BCH):
                t = cpool.tile([64, c1 - c0], BF16, tag=f"whb{i}")
                (nc.sync if i == 0 else nc.gpsimd).dma_start(
                    out=t[:], in_=whb_d[:, c0:c1])
                whb_t.append(t)
            wha_t = []
            for i, (c0, c1) in enumerate(WCH):
                t = cpool.tile([128, c1 - c0], BF16, tag=f"wha{i}")
                nc.scalar.dma_start(out=t[:], in_=wha_d[:, c0:c1])
                wha_t.append(t)

            xd2 = cpool.tile([128, PC], BF16, tag="xd2")
            nc.sync.dma_start(out=xd2[0:64, :], in_=xc_d[0:64, :])
            nc.sync.dma_start(out=xd2[64:128, :], in_=xc_d[64:128, :])

            # PE p-state ramp: the clock starts ~0.8GHz and climbs only
            # under continuous execution; keep PE busy with cheap N=128
            # matmuls until the first W chunk lands (output never read).
            wsc = cpool.tile([128, 128], BF16, tag="wsc")
            nc.vector.memset(wsc[:], 0.0)
            ywarm = ypool.tile([64, 128], F32, tag="ywarm", bufs=1)
            for w in range(N_WARM):
                nc.tensor.matmul(ywarm[:], ghA, wsc[:],
                                 start=(w == 0), stop=(w == N_WARM - 1),
                                 skip_group_check=True)

            # DMA cannot read PSUM: RED-matmul results bounce through an
            # SBUF staging buffer via ACT copies.
            ro = cpool.tile([24, PC], F32, tag="ro")

            def emit_pair(p):
                y = ypool.tile([128, 512], F32, tag="y", bufs=3)
                for h in range(2):
                    g0 = 1024 * p + 512 * h
                    wa, sa = _locate(wha_t, WCH, g0, 512)
                    nc.tensor.matmul(y[64 * h:64 * h + 64, :], ghA,
                                     wa[:, sa], start=True, stop=False,
                                     skip_group_check=True)
                wb, sb = _locate(whb_t, BCH, 512 * p, 512)
                nc.tensor.matmul(y[:], ghB2, wb[:, sb],
                                 start=False, stop=True,
                                 skip_group_check=True)
                s = spool.tile([128, 512], BF16, tag="s")
                nc.vector.tensor_mul(out=s[:], in0=y[:],
                                     in1=xd2[:, 512 * p:512 * p + 512])
                return s

            def emit_red(p, s):
                r = rpool.tile([24, 512], F32, tag="r", bufs=3)
                nc.tensor.matmul(r[:], RED24, s[:], start=True, stop=True,
                                 skip_group_check=True)
                nc.scalar.copy(out=ro[:, 512 * p:512 * p + 512], in_=r[:])
                if p % 2 == 1:
                    c0 = 1024 * (p // 2)
                    nc.sync.dma_start(out=outT[:, c0:c0 + 1024],
                                      in_=ro[:, c0:c0 + 1024])


            # software pipeline: RED(p) runs two pairs behind the A/B
            # matmuls so the PE never waits on the DVE multiply.
            s_tiles = {}
            for p in range(NPAIR):
                s_tiles[p] = emit_pair(p)
                if p >= 2:
                    emit_red(p - 2, s_tiles.pop(p - 2))

            # 128-vertex tail (single half), using pooled tile slices
            yt = ypool.tile([128, 512], F32, tag="y", bufs=3)
            wa, sa = _locate(wha_t, WCH, 6144, 128)
            wb, sb = _locate(whb_t, BCH, 3072, 128)
            nc.tensor.matmul(yt[0:64, 0:128], ghA, wa[:, sa],
                             start=True, stop=False, skip_group_check=True)
            nc.tensor.matmul(yt[0:64, 0:128], ghB2[:, 0:64], wb[:, sb],
                             start=False, stop=True, skip_group_check=True)
            st = spool.tile([128, 512], BF16, tag="s")
            nc.vector.tensor_mul(out=st[0:64, 0:128], in0=yt[0:64, 0:128],
                                 in1=xd2[0:64, 3072:3200])

            emit_red(4, s_tiles.pop(4))
            emit_red(5, s_tiles.pop(5))
            rt = rpool.tile([24, 512], F32, tag="r", bufs=3)
            nc.tensor.matmul(rt[:, 0:128], cst[0:64, 64:88], st[0:64, 0:128],
                             start=True, stop=True, skip_group_check=True)
            nc.scalar.copy(out=ro[:, 3072:3200], in_=rt[:, 0:128])
            nc.sync.dma_start(out=outT[:, 3072:3200], in_=ro[:, 3072:3200])
    nc.finalize()
    return nc


_NC_CACHE = None


def _get_nc():
    global _NC_CACHE
    if _NC_CACHE is None:
        _NC_CACHE = _build_bass()
    return _NC_CACHE


def _host_prep(X, V_nodes, rot6d_nodes, W_nodes, idx_nn_to_nodes):
    """Small per-node math (B*C=640 rows) + shard/layout of the big tensors."""
    X = np.asarray(X, np.float32)
    Vn = np.asarray(V_nodes, np.float32)
    d6 = np.asarray(rot6d_nodes, np.float32)
    W = np.asarray(W_nodes, np.float32)
    idx = np.asarray(idx_nn_to_nodes).astype(np.int64)

    a1, a2 = d6[..., :3], d6[..., 3:]
    eps = np.float32(1e-8)
    n1 = np.sqrt(np.sum(a1 * a1, -1, keepdims=True, dtype=np.float32))
    b1 = a1 / np.maximum(n1, eps)
    dot = np.sum(b1 * a2, -1, keepdims=True, dtype=np.float32)
    a2p = a2 - dot * b1
    n2 = np.sqrt(np.sum(a2p * a2p, -1, keepdims=True, dtype=np.float32))
    b2 = a2p / np.maximum(n2, eps)
    b3 = np.cross(b1, b2)
    R = np.stack([b1, b2, b3], axis=-2).astype(np.float32)  # (B,C,3,3) [b,c,k,d]

    center = X[:, idx, :]                                   # (B,C,3)
    t = (center + Vn - np.einsum('bcd,bckd->bck', center, R)).astype(np.float32)

    # G columns at j = k*16 + d*4 + b (d==3 = translation); cols 48:64 zero
    Gv = np.zeros((C, 4, 4, 4), np.float32)
    Gv[:, 0:3, 0:3, :] = np.transpose(R, (1, 2, 3, 0))
    Gv[:, 0:3, 3, :] = np.transpose(t, (1, 2, 0))
    G = Gv.reshape(C, 64)

    RED = np.zeros((2, 4, 4, 4, 24), np.float32)
    for h in range(2):
        for k in range(3):
            for b in range(B):
                RED[h, k, :, b, h * 12 + k * 4 + b] = 1.0
    RED = RED.reshape(128, 24)

    cst = np.zeros((128, 224), NPBF16)
    cst[:, 0:64] = G[0:128].astype(NPBF16)
    cst[:, 64:88] = RED.astype(NPBF16)
    gB = G[128:160].astype(NPBF16)             # [32, 64]
    cst[0:32, 96:160] = gB                     # even-half block
    cst[32:64, 160:224] = gB                   # odd-half block

    Wb = W.astype(NPBF16)
    in_maps = []
    for i in range(N_CORES):
        vsl = slice(i * VS, (i + 1) * VS)
        wt = np.zeros((160, VSP), NPBF16)
        wt[:, :VS] = Wb[vsl].T
        wha = np.ascontiguousarray(wt[0:128])
        # B part in vertex-pair-column layout: whb[(h*32+c), 512p+j] =
        # W_B[c, 1024p + 512h + j]; tail (cols 3072:3200) even-half only
        bp = wt[128:160]                       # [32, VSP]
        whb = np.zeros((64, PC), NPBF16)
        whb[:, 0:3072] = bp[:, :6144].reshape(32, NPAIR, 2, 512).transpose(
            2, 0, 1, 3).reshape(64, 3072)
        whb[0:32, 3072:3200] = bp[:, 6144:6272]

        Xs = np.zeros((B, VSP, 3), np.float32)
        Xs[:, :VS] = X[:, vsl, :]
        xc = np.zeros((2, 4, 4, PC), np.float32)        # [h, d, b, col]
        main = Xs[:, :6144].reshape(B, NPAIR, 2, 512, 3)
        xc[:, 0:3, :, 0:3072] = np.transpose(
            main, (2, 4, 0, 1, 3)).reshape(2, 3, B, 3072)
        xc[:, 3, :, 0:3072] = 1.0
        xc[0, 0:3, :, 3072:3200] = np.transpose(Xs[:, 6144:6272], (2, 0, 1))
        xc[0, 3, :, 3072:3200] = 1.0
        xc = xc.reshape(2, 16, PC)
        # ship fully inflated: 4 copies per half ([128, PC])
        xc = np.ascontiguousarray(np.concatenate(
            [xc[0]] * 4 + [xc[1]] * 4, 0).astype(NPBF16))

        in_maps.append({"cst": cst, "wha": wha, "whb": whb, "xc": xc})
    return in_maps


def _gather(results):
    out = np.empty((B, V, 3), np.float32)
    for i, res in enumerate(results):
        o = res["outT"].reshape(2, 3, 4, PC)            # [h, k, b, col]
        om = o[:, :, :, 0:3072].reshape(2, 3, 4, NPAIR, 512)
        block = np.empty((B, VSP, 3), np.float32)
        block[:, :6144] = np.transpose(om, (2, 3, 0, 4, 1)).reshape(B, 6144, 3)
        block[:, 6144:6272] = np.transpose(o[0, :, :, 3072:3200], (1, 2, 0))
        out[:, i * VS:(i + 1) * VS] = block[:, :VS]
    return out


def kernel(X, V_nodes, rot6d_nodes, W_nodes, idx_nn_to_nodes, **run_kwargs):
    in_maps = _host_prep(X, V_nodes, rot6d_nodes, W_nodes, idx_nn_to_nodes)
    res = run_bass_kernel_spmd(_get_nc(), in_maps,
                               core_ids=list(range(N_CORES)), **run_kwargs)
    out = _gather(res.results)
    kernel.last_run = res
    return out
